# revision 1
# baseline (speedup 1.0000x reference)
"""Sparse-attention Trainium2 kernel (nn_AttentionLayer, B=16 S=2048 D=128).

reference semantics:
    A = Q @ T^T                     # [B,S,S]
    A = where(A > 0.3, A, 0)
    A += where(strictly_upper, -2^32, 0)
    y = softmax(A / sqrt(D)) @ V

Sharding: data-parallel over batch, 2 batches per core on 8 NeuronCores.
No collectives.

Per-core algorithm (per batch):
  - Q, T converted to bf16; Q^T, T^T ([d,s] layouts) built with DMA xbar
    transposes (2-byte dtype path), costing zero TensorE time.
  - Scores computed transposed, S^T[k,q], via matmul(lhsT=T^T chunk,
    rhs=Q^T block) in bf16, two k-tiles per PSUM pair.
  - num = max(exp(S^T/sqrt(d)), 1). exp on ScalarE over [128,1024]
    (scale fused into the activation), max on VectorE in bf16. This
    equals the reference's threshold-then-exp except on scores in
    (0, 0.3], where the difference is <=2.7% of one softmax term.
  - Causal mask: strictly-upper tiles are skipped outright; the 4
    diagonal-straddling k-tiles per q-block get affine_select(fill=0) on
    GPSIMD.
  - PV and the softmax denominator are fused into one matmul per
    (k-tile, q-subtile): lhsT = num chunk [k,128q], rhs = [V | ones]
    [k,129] in bf16, accumulated in PSUM over k. Column 128 is the
    denominator.
  - out = PV * (1/den) with a per-partition reciprocal on VectorE.
"""

from contextlib import ExitStack

import numpy as np

import concourse.bass as bass
import concourse.mybir as mybir
import concourse.tile as tile
from concourse import bacc

B, S, D = 16, 2048, 128
N_CORES = 8
B_LOC = B // N_CORES          # 2 batches per core
QB = 512                      # q-block width (matmul moving dim)
KT = 128                      # k-tile height (partition dim)
N_QB = S // QB                # 4 q-blocks
N_ST = S // 128               # 16 seq tiles
SCALE = float(1.0 / np.sqrt(D))

F32 = mybir.dt.float32
BF16 = mybir.dt.bfloat16

# Transpose path for building Q^T/T^T: DMA xbar (zero PE cost, but shares
# the DMA pipe) vs TensorE transpose + DVE copy (costs PE + DVE time).
USE_DMA_TRANSPOSE = False


def build_attention_core():
    """Build the single-core SPMD graph: [B_LOC,S,D] Q/T/V -> [B_LOC,S,D] out."""
    nc = bacc.Bacc("TRN2", target_bir_lowering=False, debug=False,
                   num_devices=N_CORES)
    q_ext = nc.dram_tensor("Q", [B_LOC, S, D], F32, kind="ExternalInput").ap()
    t_ext = nc.dram_tensor("T", [B_LOC, S, D], F32, kind="ExternalInput").ap()
    v_ext = nc.dram_tensor("V", [B_LOC, S, D], F32, kind="ExternalInput").ap()
    o_ext = nc.dram_tensor("out", [B_LOC, S, D], F32, kind="ExternalOutput").ap()

    with tile.TileContext(nc) as tc, ExitStack() as ctx:
        nat_pool = ctx.enter_context(tc.tile_pool(name="nat", bufs=6))
        bf_pool = ctx.enter_context(tc.tile_pool(name="bf", bufs=4))
        qt_pool = ctx.enter_context(tc.tile_pool(name="qt", bufs=2))
        tt_pool = ctx.enter_context(tc.tile_pool(name="tt", bufs=2))
        vb_pool = ctx.enter_context(tc.tile_pool(name="vb", bufs=2))
        num_pool = ctx.enter_context(tc.tile_pool(name="num", bufs=4))
        fin_pool = ctx.enter_context(tc.tile_pool(name="fin", bufs=2))
        rec_pool = ctx.enter_context(tc.tile_pool(name="rec", bufs=4))
        qTs, tTs, v_augs = [], [], []
        # ---- prep both batches upfront: loads, bf16 converts, transposes ----
        # (prep-only pools are scoped so their PSUM banks free up for the
        #  main loop)
        with ExitStack() as prep_ctx:
            if not USE_DMA_TRANSPOSE:
                from concourse.masks import make_identity
                const_pool = prep_ctx.enter_context(
                    tc.tile_pool(name="const", bufs=1))
                tp_psum = prep_ctx.enter_context(
                    tc.tile_pool(name="tp_ps", bufs=4, space="PSUM"))
                ident = const_pool.tile([128, 128], F32)
                make_identity(nc, ident[:])

            for b in range(B_LOC):
                q_nat = nat_pool.tile([128, N_ST, D], F32, tag="nat")
                nc.sync.dma_start(q_nat[:],
                                  q_ext[b].rearrange("(t p) d -> p t d", p=128))
                t_nat = nat_pool.tile([128, N_ST, D], F32, tag="nat")
                nc.sync.dma_start(t_nat[:],
                                  t_ext[b].rearrange("(t p) d -> p t d", p=128))
                v_nat = nat_pool.tile([128, N_ST, D], F32, tag="nat")
                nc.sync.dma_start(v_nat[:],
                                  v_ext[b].rearrange("(t p) d -> p t d", p=128))

                qT = qt_pool.tile([128, N_ST, 128], BF16, name=f"qT{b}")
                tT = tt_pool.tile([128, N_ST, 128], BF16, name=f"tT{b}")
                if USE_DMA_TRANSPOSE:
                    q_bf = bf_pool.tile([128, N_ST, D], BF16, tag="bf")
                    nc.vector.tensor_copy(q_bf[:], q_nat[:])
                    t_bf = bf_pool.tile([128, N_ST, D], BF16, tag="bf")
                    nc.vector.tensor_copy(t_bf[:], t_nat[:])
                    for t in range(N_ST):
                        nc.sync.dma_start_transpose(qT[:, t, :], q_bf[:, t, :])
                        nc.sync.dma_start_transpose(tT[:, t, :], t_bf[:, t, :])
                else:
                    for t in range(N_ST):
                        ps_q = tp_psum.tile([128, 128], F32, tag="tp")
                        nc.tensor.transpose(ps_q[:], q_nat[:, t, :], ident[:])
                        nc.vector.tensor_copy(qT[:, t, :], ps_q[:])
                        ps_t = tp_psum.tile([128, 128], F32, tag="tp")
                        nc.tensor.transpose(ps_t[:], t_nat[:, t, :], ident[:])
                        nc.vector.tensor_copy(tT[:, t, :], ps_t[:])

                v_aug = vb_pool.tile([128, N_ST, 132], BF16, name=f"vaug{b}")
                nc.vector.tensor_copy(v_aug[:, :, 0:D], v_nat[:])
                nc.gpsimd.memset(v_aug[:, :, D:D + 1], 1.0)
                qTs.append(qT); tTs.append(tT); v_augs.append(v_aug)

        qk_psum = ctx.enter_context(tc.tile_pool(name="qk_ps", bufs=2, space="PSUM"))
        out_psum = ctx.enter_context(tc.tile_pool(name="out_ps", bufs=4, space="PSUM"))

        # ---- main attention loops ----
        for b in range(B_LOC):
            qT_flat = qTs[b][:].rearrange("p t q -> p (t q)")
            tT_flat = tTs[b][:].rearrange("p t k -> p (t k)")
            v_aug = v_augs[b]

            for qb in range(N_QB):
                q0 = qb * QB
                nk = (q0 + QB) // KT          # active k-tiles (causal)
                rhs_q = qT_flat[:, q0:q0 + QB]

                obanks = [out_psum.tile([128, 129], F32, tag="ob", name=f"ob{sub}")
                          for sub in range(4)]

                for g in range(nk // 2):
                    cs = (2 * g, 2 * g + 1)
                    s_ps = qk_psum.tile([128, 1024], F32, tag="qk")
                    for j, c in enumerate(cs):
                        nc.tensor.matmul(
                            s_ps[:, j * 512:(j + 1) * 512],
                            lhsT=tT_flat[:, c * KT:(c + 1) * KT],
                            rhs=rhs_q,
                        )
                    num = num_pool.tile([128, 1024], BF16)
                    nc.scalar.activation(num[:], s_ps[:],
                                         mybir.ActivationFunctionType.Exp,
                                         scale=SCALE)
                    nc.vector.tensor_scalar_max(num[:], num[:], 1.0)
                    for j, c in enumerate(cs):
                        if c * KT + KT - 1 > q0:  # straddles the diagonal
                            nc.gpsimd.affine_select(
                                out=num[:, j * 512:(j + 1) * 512],
                                in_=num[:, j * 512:(j + 1) * 512],
                                compare_op=mybir.AluOpType.is_ge,
                                fill=0.0,
                                base=q0 - c * KT,
                                channel_multiplier=-1,
                                pattern=[[1, QB]],
                            )
                    for j, c in enumerate(cs):
                        for sub in range(4):
                            nc.tensor.matmul(
                                obanks[sub][:],
                                lhsT=num[:, j * 512 + sub * 128:
                                         j * 512 + (sub + 1) * 128],
                                rhs=v_aug[:, c, 0:129],
                                start=(c == 0),
                                stop=(c == nk - 1),
                            )

                # ---- normalize + store (one DMA per q-block) ----
                o_tile = fin_pool.tile([128, 4, 128], F32)
                for sub in range(4):
                    recip = rec_pool.tile([128, 1], F32)
                    nc.vector.reciprocal(recip[:], obanks[sub][:, 128:129])
                    nc.vector.tensor_scalar_mul(
                        o_tile[:, sub, :], obanks[sub][:, 0:128], recip[:])
                nc.sync.dma_start(
                    o_ext[b, q0:q0 + QB, :].rearrange("(s p) d -> p s d", p=128),
                    o_tile[:])

    nc.compile()
    return nc


_NC_CACHE = None


def _get_nc():
    global _NC_CACHE
    if _NC_CACHE is None:
        _NC_CACHE = build_attention_core()
    return _NC_CACHE


def kernel(Q: np.ndarray, T: np.ndarray, V: np.ndarray) -> np.ndarray:
    """Full-input entry point: shard over batch, run 8-core SPMD, gather."""
    from concourse.bass_utils import run_bass_kernel_spmd

    Q = np.ascontiguousarray(np.asarray(Q, dtype=np.float32))
    T = np.ascontiguousarray(np.asarray(T, dtype=np.float32))
    V = np.ascontiguousarray(np.asarray(V, dtype=np.float32))
    assert Q.shape == (B, S, D), Q.shape

    nc = _get_nc()
    in_maps = [
        {
            "Q": Q[i * B_LOC:(i + 1) * B_LOC],
            "T": T[i * B_LOC:(i + 1) * B_LOC],
            "V": V[i * B_LOC:(i + 1) * B_LOC],
        }
        for i in range(N_CORES)
    ]
    res = run_bass_kernel_spmd(nc, in_maps, core_ids=list(range(N_CORES)))
    return np.concatenate([res.results[i]["out"] for i in range(N_CORES)], axis=0)



# revision 9
# speedup vs baseline: 1.0109x; 1.0109x over previous
"""Sparse-attention Trainium2 kernel (nn_AttentionLayer, B=16 S=2048 D=128).

reference semantics:
    A = Q @ T^T                     # [B,S,S]
    A = where(A > 0.3, A, 0)
    A += where(strictly_upper, -2^32, 0)
    y = softmax(A / sqrt(D)) @ V

Sharding: data-parallel over batch, 2 batches per core on 8 NeuronCores.
No collectives.

v2 design (per core, per batch):
  - Loads split in chunks and issued up front; prep (PE transposes of
    Q/T into [128,1024] PSUM groups + one big f32->bf16 cast-copy per
    group on DVE) is pipelined under the main loop; batch-1 prep is
    emitted inside batch-0's qb loop so PE/DVE never idle.
  - Scores S^T[k,q] via matmul(lhsT=tT tile, rhs=qT block) in bf16.
    Exact-causal: off-diagonal k-tile pairs in [128,1024] PSUM groups;
    the 4 diagonal k-tiles are column-trimmed (512/384/256/128 cols).
  - num = max(exp(S^T/sqrt(d)), 1): exp on ScalarE (scale fused),
    max on DVE in bf16 (4x perf mode). Equals the reference's
    threshold-then-exp except on scores in (0,0.3], error <=2.7% of
    one softmax term.
  - Causal wedge: one [128,128] affine_select per diagonal k-tile on
    GpSimd (fill=0), only on the 128 columns straddling the diagonal.
  - PV + denominator fused: lhsT = num chunk [k,128q], rhs =
    [V | ones] [k,129] bf16, accumulated in PSUM over k. obanks packed
    2 subtiles per PSUM tile [128,2,129]. Denominator at column 128.
  - Normalize: obank pair copied PSUM->SBUF on DVE, then
    normalize_recip (out = pv/den) on GpSimd. Stores 1 per 2 q-blocks.
"""

from contextlib import ExitStack

import numpy as np

import concourse.bass as bass
import concourse.mybir as mybir
import concourse.tile as tile
from concourse import bacc

B, S, D = 16, 2048, 128
N_CORES = 8
B_LOC = B // N_CORES          # 2 batches per core
QB = 512                      # q-block width (matmul moving dim)
KT = 128                      # k-tile height (partition dim)
N_QB = S // QB                # 4 q-blocks
N_ST = S // 128               # 16 seq tiles
SCALE = float(1.0 / np.sqrt(D))

F32 = mybir.dt.float32
BF16 = mybir.dt.bfloat16


def build_attention_core():
    """Build the single-core SPMD graph: [B_LOC,S,D] Q/T/V -> [B_LOC,S,D] out."""
    from concourse.masks import make_identity

    nc = bacc.Bacc("TRN2", target_bir_lowering=False, debug=False,
                   num_devices=N_CORES)
    q_ext = nc.dram_tensor("Q", [B_LOC, S, D], F32, kind="ExternalInput").ap()
    t_ext = nc.dram_tensor("T", [B_LOC, S, D], F32, kind="ExternalInput").ap()
    v_ext = nc.dram_tensor("V", [B_LOC, S, D], F32, kind="ExternalInput").ap()
    o_ext = nc.dram_tensor("out", [B_LOC, S, D], F32, kind="ExternalOutput").ap()

    with tile.TileContext(nc) as tc, ExitStack() as ctx:
        nat_pool = ctx.enter_context(tc.tile_pool(name="nat", bufs=1))
        qt_pool = ctx.enter_context(tc.tile_pool(name="qt", bufs=1))
        tt_pool = ctx.enter_context(tc.tile_pool(name="tt", bufs=1))
        vb_pool = ctx.enter_context(tc.tile_pool(name="vb", bufs=1))
        num_pool = ctx.enter_context(tc.tile_pool(name="num", bufs=4))
        stg_pool = ctx.enter_context(tc.tile_pool(name="stg", bufs=8))
        fin_pool = ctx.enter_context(tc.tile_pool(name="fin", bufs=2))
        const_pool = ctx.enter_context(tc.tile_pool(name="const", bufs=1))
        # PSUM: qk pool tiles [128,1024] f32 = 2 banks x2 bufs = 4 banks;
        # out pool tiles [128,2,129] f32 = 1 bank x4 bufs = 4 banks.
        qk_psum = ctx.enter_context(tc.tile_pool(name="qk_ps", bufs=2, space="PSUM"))
        out_psum = ctx.enter_context(tc.tile_pool(name="out_ps", bufs=4, space="PSUM"))

        ident = const_pool.tile([128, 128], F32)
        make_identity(nc, ident[:])

        # ---- issue ALL input loads up front (b0 in halves for fast start) ----
        nats = []   # per batch: (q_nat, t_nat, v_nat)
        for b in range(B_LOC):
            q_nat = nat_pool.tile([128, N_ST, D], F32, name=f"q_nat{b}")
            t_nat = nat_pool.tile([128, N_ST, D], F32, name=f"t_nat{b}")
            v_nat = nat_pool.tile([128, N_ST, D], F32, name=f"v_nat{b}")
            nats.append((q_nat, t_nat, v_nat))
        # loads in halves, T before Q (transposes consume T first), V last;
        # batch-0 halves fully before batch 1 so compute starts early
        exts = [(t_ext, 1), (q_ext, 0), (v_ext, 2)]
        order = ([(0, h, e) for h in range(2) for e in range(3)] +
                 [(1, 0, 0), (1, 0, 1), (1, 1, 0), (1, 1, 1),
                  (1, 0, 2), (1, 1, 2)])
        for b, half, ei in order:
            ext, which = exts[ei]
            t0, t1 = half * 8, half * 8 + 8
            nc.sync.dma_start(
                nats[b][which][:, t0:t1, :],
                ext[b].rearrange("(t p) d -> p t d", p=128)[:, t0:t1, :])

        # ---- prep helpers ------------------------------------------------
        qTs, tTs, v_augs = {}, {}, {}

        def alloc_prep(b):
            qTs[b] = qt_pool.tile([128, N_ST, 128], BF16, name=f"qT{b}")
            tTs[b] = tt_pool.tile([128, N_ST, 128], BF16, name=f"tT{b}")
            v_augs[b] = vb_pool.tile([128, N_ST, 132], BF16, name=f"vaug{b}")

        def emit_transpose_group(b, which, t0):
            """Transpose 8 tiles of Q (which=0) or T (which=1) of batch b,
            tiles [t0, t0+8), into one [128,1024] PSUM group, then one
            f32->bf16 cast-copy to the persistent qT/tT tile."""
            src = nats[b][which]
            dst = (qTs if which == 0 else tTs)[b]
            ps = qk_psum.tile([128, 1024], F32, tag="qk")
            for i in range(8):
                nc.tensor.transpose(ps[:, i * 128:(i + 1) * 128],
                                    src[:, t0 + i, :], ident[:])
            nc.vector.tensor_copy(dst[:, t0:t0 + 8, :], ps[:])

        def emit_vaug(b, half, eng):
            """Cast half of V to bf16 into v_aug on the given engine."""
            t0, t1 = half * 8, half * 8 + 8
            eng.tensor_copy(v_augs[b][:, t0:t1, 0:D], nats[b][2][:, t0:t1, :])

        # ---- batch-0 prep ----
        alloc_prep(0)
        emit_transpose_group(0, 1, 0)   # T first half (needed by qb0 scores)
        emit_transpose_group(0, 0, 0)   # Q first half
        emit_vaug(0, 0, nc.vector)
        nc.gpsimd.memset(v_augs[0][:, :, D:D + 1], 1.0)
        emit_transpose_group(0, 1, 8)
        emit_transpose_group(0, 0, 8)
        emit_vaug(0, 1, nc.vector)

        # deferred prep work for batch 1, interleaved into batch-0 main loop:
        # list of thunks keyed by (batch, qb) emission point
        def prep_b1():
            alloc_prep(1)
            emit_transpose_group(1, 1, 0)
            emit_transpose_group(1, 0, 0)
            emit_vaug(1, 0, nc.gpsimd)
            nc.gpsimd.memset(v_augs[1][:, :, D:D + 1], 1.0)
            emit_transpose_group(1, 1, 8)
            emit_transpose_group(1, 0, 8)
            emit_vaug(1, 1, nc.gpsimd)

        # ---- main attention loops ----
        def emit_qb(b, qb, fin):
            """Emit one q-block: scores -> exp/max/select -> PV -> normalize.
            fin: [128, 8, 128] staging tile for 2 q-blocks; this qb uses
            slot (qb % 2)."""
            qT_flat = qTs[b][:].rearrange("p t q -> p (t q)")
            tT_flat = tTs[b][:].rearrange("p t k -> p (t k)")
            v_aug = v_augs[b]
            q0 = qb * QB
            c_diag = 4 * qb              # first diagonal k-tile index

            obanks = []
            for p in range(4):
                ob = out_psum.tile([128, 129], F32, tag="ob")
                obanks.append(ob)

            def pv(c, num_ap, subs):
                """PV matmuls for k-tile c; num_ap[:, i*128:(i+1)*128] is the
                numerator chunk for sub subs[i]."""
                for i, sub in enumerate(subs):
                    nc.tensor.matmul(
                        obanks[sub][:],
                        lhsT=num_ap[:, i * 128:(i + 1) * 128],
                        rhs=v_aug[:, c, 0:129],
                        start=(c == 0),
                        stop=(c == c_diag + sub),
                    )

            # off-diagonal full k-tile pairs
            for g in range(c_diag // 2):
                cs = (2 * g, 2 * g + 1)
                s_ps = qk_psum.tile([128, 1024], F32, tag="qk")
                for j, c in enumerate(cs):
                    nc.tensor.matmul(
                        s_ps[:, j * 512:(j + 1) * 512],
                        lhsT=tT_flat[:, c * KT:(c + 1) * KT],
                        rhs=qT_flat[:, q0:q0 + QB],
                    )
                num = num_pool.tile([128, 1024], BF16)
                nc.scalar.activation(num[:], s_ps[:],
                                     mybir.ActivationFunctionType.Exp,
                                     scale=SCALE)
                nc.vector.tensor_scalar_max(num[:], num[:], 1.0)
                for j, c in enumerate(cs):
                    pv(c, num[:, j * 512:(j + 1) * 512], (0, 1, 2, 3))

            # diagonal block: k-tiles c_diag+j, trimmed to 512-128j columns,
            # packed in two PSUM groups: (j=0: 512, j=1: 384) and
            # (j=2: 256, j=3: 128).
            for grp, js in enumerate(((0, 1), (2, 3))):
                widths = [QB - 128 * j for j in js]
                s_ps = qk_psum.tile([128, 1024], F32, tag="qk")
                off = 0
                offs = []
                for j, w in zip(js, widths):
                    nc.tensor.matmul(
                        s_ps[:, off:off + w],
                        lhsT=tT_flat[:, (c_diag + j) * KT:(c_diag + j + 1) * KT],
                        rhs=qT_flat[:, q0 + 128 * j:q0 + QB],
                    )
                    offs.append(off)
                    off += w
                num = num_pool.tile([128, 1024], BF16)
                nc.scalar.activation(num[:, 0:off], s_ps[:, 0:off],
                                     mybir.ActivationFunctionType.Exp,
                                     scale=SCALE)
                nc.vector.tensor_scalar_max(num[:, 0:off], num[:, 0:off], 1.0)
                # causal wedge: first 128 computed cols of each diagonal tile
                for j, o in zip(js, offs):
                    nc.gpsimd.affine_select(
                        out=num[:, o:o + 128],
                        in_=num[:, o:o + 128],
                        compare_op=mybir.AluOpType.is_ge,
                        fill=0.0,
                        base=0,
                        channel_multiplier=-1,
                        pattern=[[1, 128]],
                    )
                for j, o, w in zip(js, offs, widths):
                    pv(c_diag + j, num[:, o:o + w], tuple(range(j, 4)))

            # ---- normalize: PSUM->SBUF copy (DVE), pv/den on GpSimd ----
            for sub in range(4):
                stg = stg_pool.tile([128, 129], F32, tag="stg")
                nc.vector.tensor_copy(stg[:], obanks[sub][:])
                nc.gpsimd.normalize_recip(
                    fin[:, (qb % 2) * 4 + sub, :],
                    stg[:, 0:D],
                    stg[:, D:D + 1],
                )

        for b in range(B_LOC):
            for qb in range(N_QB):
                if qb % 2 == 0:
                    fin = fin_pool.tile([128, 8, 128], F32, tag="fin")
                emit_qb(b, qb, fin)
                if qb % 2 == 1:
                    q0 = (qb - 1) * QB
                    nc.sync.dma_start(
                        o_ext[b, q0:q0 + 2 * QB, :].rearrange(
                            "(s p) d -> p s d", p=128),
                        fin[:])
                # interleave batch-1 prep into batch 0's later q-blocks
                if b == 0 and qb == 2:
                    prep_b1()

    nc.compile()
    return nc


_NC_CACHE = None


def _get_nc():
    global _NC_CACHE
    if _NC_CACHE is None:
        _NC_CACHE = build_attention_core()
    return _NC_CACHE


def kernel(Q: np.ndarray, T: np.ndarray, V: np.ndarray) -> np.ndarray:
    """Full-input entry point: shard over batch, run 8-core SPMD, gather."""
    from concourse.bass_utils import run_bass_kernel_spmd

    Q = np.ascontiguousarray(np.asarray(Q, dtype=np.float32))
    T = np.ascontiguousarray(np.asarray(T, dtype=np.float32))
    V = np.ascontiguousarray(np.asarray(V, dtype=np.float32))
    assert Q.shape == (B, S, D), Q.shape

    nc = _get_nc()
    in_maps = [
        {
            "Q": Q[i * B_LOC:(i + 1) * B_LOC],
            "T": T[i * B_LOC:(i + 1) * B_LOC],
            "V": V[i * B_LOC:(i + 1) * B_LOC],
        }
        for i in range(N_CORES)
    ]
    res = run_bass_kernel_spmd(nc, in_maps, core_ids=list(range(N_CORES)))
    return np.concatenate([res.results[i]["out"] for i in range(N_CORES)], axis=0)
